# revision 3
# baseline (speedup 1.0000x reference)
"""GCN layer (gather + segment_sum + scale) on 8 Trainium2 NeuronCores.

Strategy (1D destination-node parallel, dma_gather based):
  - Host (integer/index/layout work only): shard edges by dst block of 12500
    nodes. Per core, sources are 4-colored independently (per-core table
    content!), balancing each destination's in-edges across colors with
    nested remainder caps. The flat DRAM table packs the 4 color classes as
    64B sub-slots of one 256B-stride row (row r holds positions 4r..4r+3),
    so idx = class position fits int16 and the table write is contiguous.
    Nodes are packed into 98 groups of 128 ranks by a greedy that minimizes
    per-color degree maxima; group profiles are max-shared across cores so
    one compiled program serves all 8 cores.
  - Device (all FP math): build the prescaled bf16 table h = node_f*out_d in
    DRAM via a 4-chunk load->mult->write pipeline (each chunk's write is a
    fully contiguous 1.6MB DMA), then one dma_gather per <=112 columns
    (64B payloads). The Tensor engine accumulates slot columns into PSUM
    (fp32) via identity matmuls; finalize multiplies by in_dg on DVE and
    DMAs out in bf16.
  - Host: inverse-permute rows back to original node order, widen to fp32.
"""

import sys

import numpy as np

for _p in ("/opt/trn_rl_repo", "/root/.axon_site/_ro/trn_rl_repo"):
    if _p not in sys.path:
        sys.path.append(_p)

P = 128
D = 32
N_CORES = 8
N = 100000
NPC = N // N_CORES            # 12500 dst nodes per core
G = (NPC + P - 1) // P        # 98 groups
RANKS = G * P                 # 12544
Q = 4                         # color classes (64B sub-slots of a 256B row)
RPB = 196                     # table rows per SBUF partition block
R_ROWS = P * RPB              # 25088 table rows (int16-addressable)
PAD_IDX = R_ROWS - 1          # last row: positions 100348..100351, always 0
XPP = 4 * RPB                 # 784 positions per SBUF partition
NPOS = P * XPP                # 100352 table positions
TW = 128                      # table row width in bf16 elems (256B stride)
TILE_COLS = 112               # max columns per dma_gather (14336 idxs)
BANK = 16                     # psum groups per bank
NB = (G + BANK - 1) // BANK   # 7 banks
N_CHUNK = 4                   # table build pipeline chunks

_cache = {}


# ---------------------------------------------------------------- host prep


def _color_core(e_src, e_dstl):
    """Per-core source coloring, balancing each dst's in-edges across the 4
    colors (nested remainder caps). Returns col_u [N] int8 (-1 = unplaced).
    """
    deg = np.bincount(e_dstl, minlength=NPC).astype(np.int32)
    # nested remainder caps (remainder edges on the low colors, same pattern
    # for every dst): profiles become monotone in degree, so a group's
    # per-color maxima sum to exactly its max degree — no color penalty.
    # The class-size skew this causes is repaired after the singles pass.
    cap = (deg[:, None] // Q
           + (np.arange(Q, dtype=np.int32)[None, :] < (deg[:, None] % Q))
           ).astype(np.int32)

    es = np.argsort(e_src, kind="stable")
    s_sorted = e_src[es]
    d_by_src = e_dstl[es]
    us, starts = np.unique(s_sorted, return_index=True)
    cnt = np.diff(np.r_[starts, len(s_sorted)])

    deg_q = np.zeros((NPC, Q), np.int32)
    col_u = np.full(N, -1, np.int8)
    class_sz = np.zeros(Q, np.int64)
    CAPQ = R_ROWS - 1  # class capacity (stride rows minus the pad row)

    multi = np.nonzero(cnt >= 2)[0]
    multi = multi[np.argsort(-cnt[multi], kind="stable")]
    for k in multi:
        vs = d_by_src[starts[k]:starts[k] + cnt[k]]
        dq = deg_q[vs]
        over = np.maximum(dq + 1 - cap[vs], 0).sum(axis=0).astype(np.int64)
        over = over * (1 << 20) + class_sz
        over[class_sz >= CAPQ] = 1 << 60
        q = int(over.argmin())
        col_u[us[k]] = q
        class_sz[q] += 1
        deg_q[vs, q] += 1
    # refinement sweep: remove-and-recolor each multi source with full
    # knowledge of the final counters (cuts per-dst cap overflows ~3x)
    for k in multi:
        u = us[k]
        vs = d_by_src[starts[k]:starts[k] + cnt[k]]
        q0 = int(col_u[u])
        deg_q[vs, q0] -= 1
        class_sz[q0] -= 1
        dq = deg_q[vs]
        over = np.maximum(dq + 1 - cap[vs], 0).sum(axis=0).astype(np.int64)
        over = over * (1 << 20) + class_sz
        over[class_sz >= CAPQ] = 1 << 60
        q = int(over.argmin())
        col_u[u] = q
        class_sz[q] += 1
        deg_q[vs, q] += 1

    # degree-1 sources: per-dst vectorized fill of the residual caps
    sk = np.nonzero(cnt == 1)[0]
    if len(sk):
        sv = d_by_src[starts[sk]]
        o2 = np.argsort(sv, kind="stable")
        sv_s = sv[o2]
        su_s = us[sk][o2]
        b = np.r_[0, np.nonzero(np.diff(sv_s))[0] + 1]
        rl = np.diff(np.r_[b, len(sv_s)])
        rank = np.arange(len(sv_s)) - np.repeat(b, rl)
        res = np.maximum(cap[sv_s] - deg_q[sv_s], 0)
        csum = np.cumsum(res, axis=1)
        qcol = (rank[:, None] >= csum).sum(axis=1)
        over = qcol >= Q
        if over.any():  # residuals exhausted (cap overflow upstream): spread
            qcol[over] = rank[over] % Q
        # class-quota repair: nested fill overloads the low colors; demote
        # the excess singles (one per dst where possible) to later colors
        for q in range(Q):
            n_q = int((qcol == q).sum()) + int(class_sz[q])
            if n_q <= CAPQ:
                continue
            excess = np.nonzero(qcol == q)[0][CAPQ - int(class_sz[q]):]
            qcol[excess] = q + 1 if q + 1 < Q else Q - 1
        col_u[su_s] = qcol.astype(np.int8)
        np.add.at(class_sz, qcol, 1)
    assert class_sz.max() <= CAPQ, class_sz
    return col_u


def _group_nodes(deg_ch):
    """Pack dsts into groups of P: sort by effective degree
    d_eff = max_q (4*a_q + q - 3), the smallest nested-cap ladder covering
    the profile. All cores sort by the same canonical scalar, so the
    per-rank profiles align across cores and the shared max stays tight.
    """
    d_eff = (4 * deg_ch + np.arange(Q)[None, :] - 3).max(axis=1)
    order = np.lexsort(tuple(-deg_ch[:, q] for q in range(Q)) + (-d_eff,))
    pad = np.zeros((RANKS - NPC, Q), np.int64)
    gmax = np.vstack([deg_ch[order], pad]).reshape(G, P, Q).max(axis=1)
    assign = np.empty(NPC, np.int64)
    assign[order] = np.arange(NPC) // P

    rank_to_node = np.full(RANKS, -1, np.int64)
    posg = np.zeros(G, np.int64)
    for v in range(NPC):
        g = assign[v]
        rank_to_node[g * P + posg[g]] = v
        posg[g] += 1
    return rank_to_node, gmax


def _preprocess(node_f, out_d, in_dg, src, dst):
    src = src.astype(np.int64)
    dst = dst.astype(np.int64)
    core_of = dst // NPC

    per_core = []
    k_sorted_all = []
    for i in range(N_CORES):
        m = core_of == i
        e_src = src[m]
        e_dstl = dst[m] - i * NPC
        col_u = _color_core(e_src, e_dstl)
        e_ch = col_u[e_src].astype(np.int64)
        deg_ch = np.zeros((NPC, Q), np.int64)
        for c in range(Q):
            deg_ch[:, c] = np.bincount(e_dstl[e_ch == c], minlength=NPC)
        rank_to_node, k = _group_nodes(deg_ch)
        gorder = np.argsort(-k.sum(axis=1), kind="stable")
        k_sorted = k[gorder]
        rank_to_node = rank_to_node.reshape(G, P)[gorder].reshape(RANKS)
        per_core.append((e_src, e_dstl, e_ch, rank_to_node, col_u))
        k_sorted_all.append(k_sorted)

    k_shared = np.maximum.reduce(k_sorted_all)      # [G, Q]
    dead = k_shared.sum(axis=1) == 0                # all-zero group: psum init
    k_shared[dead, 0] = 1

    # per-class column sequences: (j, g) order; last class bank-major so the
    # 7 bank finalizes stagger across its span instead of piling at the end
    seq_ch = {}
    for ch in range(Q):
        seq = []
        kmax = int(k_shared[:, ch].max())
        if ch == Q - 1:
            for b in range(NB):
                glo, ghi = b * BANK, min((b + 1) * BANK, G)
                for j in range(kmax):
                    for g in range(glo, ghi):
                        if k_shared[g, ch] > j:
                            seq.append((j, g))
        else:
            for j in range(kmax):
                for g in range(G):
                    if k_shared[g, ch] > j:
                        seq.append((j, g))
        seq_ch[ch] = seq
    C_ch = [len(seq_ch[ch]) for ch in range(Q)]
    C_tot = sum(C_ch)

    # carve tiles in execution order. The first tile of class ch reads only
    # table rows r with r%RPB < (RC_CHUNK*(ch+1)) (sources constrained to the
    # early region below), so its gather depends only on the first ch+1 table
    # write chunks and its desc-gen overlaps the rest of the table build.
    # bank-aligned (multiples of BANK) so each bank's first-touch segment
    # covers the bank's whole used width with start=True
    EARLY_RC = [48, 80, TILE_COLS, TILE_COLS]
    takes = []                     # (ch, ncols, dep_chunks)
    taken = [0] * Q
    for ch in range(Q):
        n = min(EARLY_RC[ch], C_ch[ch])
        takes.append((ch, n, N_CHUNK))
        taken[ch] = n
    for ch in range(Q):
        while taken[ch] < C_ch[ch]:
            n = min(TILE_COLS, C_ch[ch] - taken[ch])
            takes.append((ch, n, N_CHUNK))
            taken[ch] += n

    # global column positions by carve order + per-tile metadata
    colpos = {ch: np.full((max(int(k_shared[:, ch].max()), 1), G), -1,
                          np.int64) for ch in range(Q)}
    cols = []                      # global order: (ch, j, g)
    tile_ch, tile_rc, tile_dep = [], [], []
    cursor = [0] * Q
    for ch, n, depc in takes:
        for j, g in seq_ch[ch][cursor[ch]:cursor[ch] + n]:
            colpos[ch][j, g] = len(cols)
            cols.append((ch, j, g))
        cursor[ch] += n
        tile_ch.append(ch)
        tile_rc.append(n)
        tile_dep.append(depc)
    n_tiles = len(tile_ch)

    # idx arrays: wrapped 16-partition layout per tile, replicated across
    # the 8 Q7 cores; per-core table layouts/contents
    idx_all = np.empty((N_CORES, 128, C_tot * 8), dtype=np.int16)
    indg_all = np.zeros((N_CORES, P, G, 1), dtype=np.float32)
    nf_all = np.zeros((N_CORES, NPOS, D), dtype=np.float32)
    od_all = np.zeros((N_CORES, NPOS, 1), dtype=np.float32)
    for i in range(N_CORES):
        e_src, e_dstl, e_ch, rank_to_node, col_u = per_core[i]
        rank_of = np.full(NPC, -1, np.int64)
        real = rank_to_node >= 0
        rank_of[rank_to_node[real]] = np.nonzero(real)[0]
        r_e = rank_of[e_dstl]
        key = r_e * Q + e_ch
        perm = np.argsort(key, kind="stable")
        ks = key[perm]
        starts = np.r_[0, np.nonzero(np.diff(ks))[0] + 1]
        runlen = np.diff(np.r_[starts, len(ks)])
        j_sorted = np.arange(len(ks)) - np.repeat(starts, runlen)
        j_e = np.empty(len(ks), np.int64)
        j_e[perm] = j_sorted
        col_l = np.empty(len(ks), np.int64)
        for ch in range(Q):
            mm = e_ch == ch
            col_l[mm] = colpos[ch][j_e[mm], r_e[mm] // P]
        assert (col_l >= 0).all()

        # class positions: sources of class ch's early tile go to the first
        # dep_chunks * (R_ROWS/N_CHUNK) table rows (a linear prefix — table
        # write chunks are partition ranges), so that tile's gather only
        # waits on those chunks
        RPCH = R_ROWS // N_CHUNK
        cpos = np.full(N, -1, np.int64)
        tile_lo = np.concatenate([[0], np.cumsum(tile_rc)])
        pad_early = [PAD_IDX] * Q  # per-class pad row for the early tiles
        for ch in range(Q):
            mem = np.nonzero(col_u == ch)[0]
            t_early = ch  # tiles 0..Q-1 are the per-class early tiles
            depc = tile_dep[t_early]
            if depc < N_CHUNK:
                in_early = (col_l[e_ch == ch] >= tile_lo[t_early]) & \
                           (col_l[e_ch == ch] < tile_lo[t_early + 1])
                s_early = np.unique(e_src[e_ch == ch][in_early])
                n_e = len(s_early)
                assert n_e + 1 <= RPCH * depc, (ch, n_e)
                cpos[s_early] = np.arange(n_e)
                # reserve row n_e (zero content) as the early tile's pad —
                # its reads must stay inside the first depc write chunks
                pad_early[ch] = n_e
                others = mem[~np.isin(mem, s_early, assume_unique=False)]
                cpos[others] = np.arange(n_e + 1, n_e + 1 + len(others))
            else:
                cpos[mem] = np.arange(len(mem))
        slots = np.full(C_tot * 128, PAD_IDX, dtype=np.int16)
        for t in range(Q):  # early tiles: class-specific pad row
            if pad_early[tile_ch[t]] != PAD_IDX:
                slots[tile_lo[t] * 128:tile_lo[t + 1] * 128] = \
                    pad_early[tile_ch[t]]
        lin = col_l * 128 + (r_e % P)
        slots[lin] = cpos[e_src].astype(np.int16)
        # wrapped 16-partition layout per tile
        wr = idx_all[i]
        for t in range(len(tile_rc)):
            blk = slots[tile_lo[t] * 128:tile_lo[t + 1] * 128]
            w = blk.reshape(-1, 16).T            # [16, rc*8]
            off = tile_lo[t] * 8
            for r2 in range(8):
                wr[16 * r2:16 * (r2 + 1), off:off + w.shape[1]] = w
        rr = np.arange(RANKS)
        indg_all[i, rr[real] % P, rr[real] // P, 0] = in_dg[
            i * NPC + rank_to_node[real], 0
        ]
        # flat table positions: pos = 4*cpos + color (pure layout)
        placed = np.nonzero(col_u >= 0)[0]
        pos_glob = 4 * cpos[placed] + col_u[placed]
        assert (cpos[placed] >= 0).all()
        nf_all[i, pos_glob] = node_f[placed]
        od_all[i, pos_glob] = out_d[placed]

    # matmul segments: (tile, col_start_in_tile, width, g_start, start_flag)
    # built in execution order. PSUM accumulation state is tracked per 2KB
    # zero region (= one bank): the FIRST segment touching a bank carries
    # start=True, which zeroes the whole bank; every later segment (any
    # column) accumulates.
    tile_lo = np.concatenate([[0], np.cumsum(tile_rc)])
    tile_of_col = np.searchsorted(tile_lo, np.arange(C_tot), side="right") - 1
    bank_started = np.zeros(NB, bool)
    touched = np.zeros(G, bool)
    segs = []
    prev = None
    for posn, (ch, j, g) in enumerate(cols):
        t = int(tile_of_col[posn])
        cs = posn - int(tile_lo[t])
        st = not bank_started[g // BANK]
        bank_started[g // BANK] = True
        touched[g] = True
        if (
            prev is not None
            and prev[4] == ch
            and prev[5] == j
            and g == prev[3] + prev[2]
            and t == prev[0]
            and g // BANK == prev[3] // BANK
        ):
            prev[2] += 1
        else:
            if prev is not None:
                segs.append(tuple(prev))
            prev = [t, cs, 1, g, ch, j, st]
    if prev is not None:
        segs.append(tuple(prev))
    assert touched.all()

    return dict(
        nf=nf_all,
        od=od_all,
        idx=idx_all,
        indg=indg_all,
        rank_to_node=[pc[3] for pc in per_core],
        segs=segs,
        C_tot=C_tot,
        tile_ch=tile_ch,
        tile_rc=tile_rc,
        tile_dep=tile_dep,
        slots_real=int(len(src)),
        slots_total=int(C_tot * 128 * N_CORES),
    )


# ---------------------------------------------------------------- device


def _build_nc(C_tot, tile_ch, tile_rc, tile_dep, segs):
    import concourse.bass as bass  # noqa: F401
    import concourse.tile as tile
    from concourse import bacc, mybir

    nc = bacc.Bacc(
        "TRN2", target_bir_lowering=False, debug=False,
        dynamic_dma_scratch_size=32768,
    )
    nf_d = nc.dram_tensor(
        "nf", [128, XPP, D], mybir.dt.float32, kind="ExternalInput"
    ).ap()
    od_d = nc.dram_tensor(
        "od", [128, XPP, 1], mybir.dt.float32, kind="ExternalInput"
    ).ap()
    idx_d = nc.dram_tensor(
        "idx", [128, C_tot * 8], mybir.dt.int16, kind="ExternalInput"
    ).ap()
    indg_d = nc.dram_tensor(
        "indg", [P, G, 1], mybir.dt.float32, kind="ExternalInput"
    ).ap()
    out_dram = nc.dram_tensor(
        "out", [P, G, D], mybir.dt.bfloat16, kind="ExternalOutput"
    ).ap()

    def raw_dma_gather(out_ap, in_ap, idxs_ap, num_idxs, elem_size, elem_step):
        eng = nc.gpsimd
        stride_bytes = elem_step * mybir.dt.size(in_ap.dtype)
        _in_ap = eng.lower_ap_dma(in_ap, for_custom_bir_dma=True)
        return eng.add_instruction(
            mybir.InstDMAGatherAnt(
                name=eng.bass.get_next_instruction_name(),
                ins=[*_in_ap, eng.lower_ap(idxs_ap),
                     eng.lower_val_access(eng.to_reg(num_idxs))],
                outs=[eng.lower_ap(out_ap)],
                transpose=False, num_idxs=num_idxs, elem_size=elem_size,
                stride_bytes_256=stride_bytes // 256, gen_mode=0,
                single_packet=False, queue_num=0, sbuf_tokens_per_rank=0,
                sbuf_free_dim_per_rank=0, sbuf_free_dim_pad_per_rank=0,
                sbuf_byte_offset=0,
            )
        )

    last_of_bank = {}
    for si, s in enumerate(segs):
        last_of_bank[s[3] // BANK] = si
    segs_by_tile = {}
    for si, s in enumerate(segs):
        segs_by_tile.setdefault(s[0], []).append((si, s))
    n_tiles = len(tile_rc)
    tile_lo = [0]
    for rc in tile_rc:
        tile_lo.append(tile_lo[-1] + rc)

    with tile.TileContext(nc) as tc:
        with (
            tc.tile_pool(name="persist", bufs=1) as persist,
            tc.tile_pool(name="build", bufs=1) as build_pool,
            tc.tile_pool(name="tabp", bufs=1, space="DRAM") as tab_pool,
            tc.tile_pool(name="msgs", bufs=4) as msgs_pool,
            tc.psum_pool(name="psum", bufs=1) as psum_pool,
        ):
            # od first (gates the first prescale); idx on SP's HWDGE so its
            # desc-gen doesn't occupy the Pool engine. The early tiles' idx
            # range loads first so their gathers unblock immediately.
            od_sb = build_pool.tile([128, XPP, 1], mybir.dt.bfloat16)
            nc.gpsimd.dma_start(out=od_sb[:], in_=od_d[:])
            idx_t = persist.tile([128, C_tot * 8], mybir.dt.int16)
            CA = tile_lo[min(6, n_tiles)] * 8
            nc.sync.dma_start(out=idx_t[:, 0:CA], in_=idx_d[:, 0:CA])
            indg_t = persist.tile([P, G, 1], mybir.dt.float32)
            nc.sync.dma_start(out=indg_t[:], in_=indg_d[:])
            if CA < C_tot * 8:
                nc.sync.dma_start(out=idx_t[:, CA:], in_=idx_d[:, CA:])

            it = persist.tile([P, P], mybir.dt.int32)
            nc.gpsimd.iota(it[:], pattern=[[1, P]], base=0, channel_multiplier=-1)
            ident = persist.tile([P, P], mybir.dt.bfloat16)
            nc.vector.tensor_scalar(
                out=ident[:], in0=it[:], scalar1=0, scalar2=None,
                op0=mybir.AluOpType.is_equal,
            )

            # table build: x-chunked load + prescale (128-wide DVE), then
            # partition-range writes so each chunk is a linear row prefix
            # of the table (gather addressing is base + idx*256 linear)
            tab = tab_pool.tile([128, RPB, TW], mybir.dt.bfloat16, name="tab")
            h_sb = build_pool.tile([128, XPP, D], mybir.dt.bfloat16)
            NXC = 8
            XCF = XPP // NXC
            for c in range(NXC):
                nc.gpsimd.dma_start(
                    out=h_sb[:, c * XCF:(c + 1) * XCF, :],
                    in_=nf_d[:, c * XCF:(c + 1) * XCF, :],
                )
            for c in range(NXC):
                nc.vector.tensor_tensor(
                    out=h_sb[:, c * XCF:(c + 1) * XCF, :],
                    in0=h_sb[:, c * XCF:(c + 1) * XCF, :],
                    in1=od_sb[:, c * XCF:(c + 1) * XCF, :].to_broadcast(
                        [128, XCF, D]),
                    op=mybir.AluOpType.mult,
                )
            PC = 128 // N_CHUNK
            for c in range(N_CHUNK):
                nc.sync.dma_start(
                    out=tab[c * PC:(c + 1) * PC, :, :],
                    in_=h_sb[c * PC:(c + 1) * PC, :, :],
                )
            pt = psum_pool.tile([P, NB * BANK, D], mybir.dt.float32)
            res = persist.tile([P, G, D], mybir.dt.bfloat16)

            for t in range(n_tiles):
                ch = tile_ch[t]
                rc = tile_rc[t]
                if rc == 0:
                    continue
                m = msgs_pool.tile([P, TILE_COLS, D], mybir.dt.bfloat16,
                                   tag="m")
                # early tiles read only the first tile_dep[t] write chunks
                # (a linear table-row prefix); the narrowed AP keeps their
                # desc-gen off the build's tail
                dep_parts = (128 // N_CHUNK) * tile_dep[t]
                raw_dma_gather(
                    m[:, 0:rc, :],
                    tab[0:dep_parts, :, 32 * ch:32 * ch + D],
                    idx_t[:, tile_lo[t] * 8:tile_lo[t] * 8 + rc * 8],
                    rc * 128, D, TW,
                )
                for si, (tt, cs, w, gs, sch, j, st) in segs_by_tile.get(t, []):
                    nc.tensor.matmul(
                        pt[:, gs:gs + w, :],
                        ident[:],
                        m[:, cs:cs + w, :],
                        start=st,
                        stop=(last_of_bank[gs // BANK] == si),
                    )
                    b = gs // BANK
                    if last_of_bank[b] == si:
                        g0 = b * BANK
                        wb = min(BANK, G - g0)
                        nc.vector.tensor_tensor(
                            out=res[:, g0:g0 + wb, :],
                            in0=pt[:, g0:g0 + wb, :],
                            in1=indg_t[:, g0:g0 + wb, :].to_broadcast(
                                [P, wb, D]),
                            op=mybir.AluOpType.mult,
                        )
                        nc.sync.dma_start(
                            out=out_dram[:, g0:g0 + wb, :],
                            in_=res[:, g0:g0 + wb, :],
                        )
    nc.compile()
    return nc


# ---------------------------------------------------------------- entry


last_run_info = {}


def kernel(node_f, out_d, in_dg, src, dst, *, _trace=False):
    node_f = np.asarray(node_f, dtype=np.float32)
    out_d = np.asarray(out_d, dtype=np.float32)
    in_dg = np.asarray(in_dg, dtype=np.float32)
    src = np.asarray(src)
    dst = np.asarray(dst)

    pp = _preprocess(node_f, out_d, in_dg, src, dst)

    key = (pp["C_tot"], len(pp["segs"]), tuple(pp["tile_rc"]))
    if key not in _cache:
        _cache.clear()
        _cache[key] = _build_nc(
            pp["C_tot"], pp["tile_ch"], pp["tile_rc"], pp["tile_dep"],
            pp["segs"]
        )
    nc = _cache[key]

    from concourse.bass_utils import run_bass_kernel_spmd

    in_maps = [
        {
            "nf": pp["nf"][i].reshape(128, XPP, D),
            "od": pp["od"][i].reshape(128, XPP, 1),
            "idx": pp["idx"][i],
            "indg": pp["indg"][i],
        }
        for i in range(N_CORES)
    ]
    # Sacrificial device touch: after an earlier crashed session the first
    # device interaction can report NRT_EXEC_UNIT_UNRECOVERABLE once and
    # then recover; absorb that here instead of failing the real run.
    try:
        import jax
        import jax.numpy as jnp

        jnp.zeros((2,)).block_until_ready()
    except Exception:
        pass

    trace_kwargs = (
        dict(trace=True, trace_cores=list(range(N_CORES))) if _trace else {}
    )
    res = None
    for attempt in range(3):
        try:
            res = run_bass_kernel_spmd(
                nc, in_maps, core_ids=list(range(N_CORES)), **trace_kwargs
            )
            break
        except ModuleNotFoundError:
            trace_kwargs = {}
        except Exception:  # noqa: BLE001
            import time as _time

            _time.sleep(2.0)
    if res is None:
        res = run_bass_kernel_spmd(nc, in_maps, core_ids=list(range(N_CORES)))
    last_run_info["exec_time_ns"] = res.exec_time_ns
    last_run_info["mean_exec_time_ns"] = res.mean_exec_time_ns
    last_run_info["trace"] = res.instructions_and_trace
    last_run_info["pp_stats"] = {
        k: pp[k] for k in ("C_tot", "slots_real", "slots_total")
    }

    out = np.empty((N, D), dtype=np.float32)
    rr = np.arange(RANKS)
    for i in range(N_CORES):
        o = np.asarray(res.results[i]["out"]).astype(np.float32)  # [P, G, D]
        r2n = pp["rank_to_node"][i]
        real = r2n >= 0
        out[i * NPC + r2n[real]] = o[rr[real] % P, rr[real] // P]
    return out
